# revision 15
# baseline (speedup 1.0000x reference)
"""ConCare forward pass distributed across 8 Trainium2 NeuronCores.

Sharding: data-parallel over batch B=64 (8 samples per core). The GRU scan,
per-feature attention, MHA and FFN are batch-local; DeCov's batch-mean and
covariance are the only cross-core terms (all-reduce via psum).
"""

import numpy as np
import jax
import jax.numpy as jnp
from jax.sharding import Mesh, PartitionSpec as P, NamedSharding

try:
    from jax import shard_map as _shard_map_fn  # jax >= 0.6
except ImportError:
    from jax.experimental.shard_map import shard_map as _shard_map_fn

B, T, F, H, A, DFF, DEMO, NH = 64, 96, 48, 128, 8, 512, 12, 4
DM = H
NCORES = 8

_ARGNAMES = [
    'x', 'demo', 'gru_Wih', 'gru_Whh', 'gru_bih', 'gru_bhh', 'att_Wt', 'att_Wx',
    'att_rate', 'demoW', 'demob', 'mWq', 'mbq', 'mWk', 'mbk', 'mWv', 'mbv',
    'mWo', 'mbo', 'ln1_a', 'ln1_b', 'ln2_a', 'ln2_b', 'ffn_W1', 'ffn_b1',
    'ffn_W2', 'ffn_b2', 'fWq', 'fbq', 'fWk', 'fbk', 'fWv', 'fbv', 'outW', 'outb',
]
_SHARDED = {'x', 'demo'}  # leading dim B; everything else replicated
_GRUNAMES = ['x', 'gru_Wih', 'gru_Whh', 'gru_bih', 'gru_bhh']
_ATTNAMES = ['att_Wt', 'att_Wx', 'att_rate']
_HEADNAMES = [n for n in _ARGNAMES
              if n not in _GRUNAMES and n not in _ATTNAMES]  # demo, mha, ffn, final


def _linear(x, W, b):
    return jnp.einsum('...i,oi->...o', x, W) + b


def _layernorm(x, a, b, eps=1e-7):
    mean = x.mean(-1, keepdims=True)
    std = jnp.std(x, axis=-1, keepdims=True, ddof=1)
    return a * (x - mean) / (std + eps) + b


def _gru_local(x, gru_Wih, gru_Whh, gru_bih, gru_bhh):
    """GRU scan on one core. x: [b,T,F] -> Hs [b,T,F,H]."""
    Bs = x.shape[0]
    Hd = gru_Whh.shape[-1]

    xT = jnp.transpose(x, (1, 2, 0))  # [T, F, b]

    def step(h, xt):  # h: [F,b,H], xt: [F,b]
        gi = xt[:, :, None] * gru_Wih[:, None, :] + gru_bih[:, None, :]  # [F,b,3H]
        gh = jax.lax.dot_general(h, gru_Whh, (((2,), (2,)), ((0,), (0,))))
        gh = gh + gru_bhh[:, None, :]  # [F,b,3H]
        ir, iz, inn = jnp.split(gi, 3, -1)
        hr, hz, hn = jnp.split(gh, 3, -1)
        r = jax.nn.sigmoid(ir + hr)
        z = jax.nn.sigmoid(iz + hz)
        n = jnp.tanh(inn + r * hn)
        hnew = (1.0 - z) * n + z * h
        return hnew, hnew

    h0 = jnp.zeros((F, Bs, Hd), x.dtype)
    _, Hs = jax.lax.scan(step, h0, xT)        # [T,F,b,H]
    return jnp.transpose(Hs, (2, 0, 1, 3))    # [b,T,F,H]


def _att_local(Hs, att_Wt, att_Wx, att_rate):
    """Per-feature time attention. Hs: [b,T,F,H] -> v [b,F,H]."""
    HsF = jnp.transpose(Hs, (2, 0, 1, 3))  # [F, b, T, H]
    hlast = HsF[:, :, -1, :]               # [F, b, H]

    # q[f,b,a] ; k[f,b,t,a]
    q = jax.lax.dot_general(hlast, att_Wt, (((2,), (1,)), ((0,), (0,))))
    HsFlat = HsF.reshape(F, -1, H)         # [F, b*T, H]
    k = jax.lax.dot_general(HsFlat, att_Wx, (((2,), (1,)), ((0,), (0,))))
    k = k.reshape(F, HsF.shape[1], T, A)   # [F, b, T, A]

    # dot[f,b,t] = sum_a q[f,b,a] * k[f,b,t,a]
    dot = jax.lax.dot_general(k, q, (((3,), (2,)), ((0, 1), (0, 1))))  # [F,b,T]
    b_time = jnp.arange(T, 0, -1, dtype=Hs.dtype)[None, None, :]
    sr = jax.nn.sigmoid(att_rate)[:, None, None]
    sd = jax.nn.sigmoid(dot)
    denom = sr * (jnp.log(2.72 + (1.0 - sd)) * b_time)
    e = jax.nn.relu(sd / denom)
    a = jax.nn.softmax(e, axis=2)          # softmax over t
    # v[f,b,h] = sum_t a[f,b,t] * Hs[f,b,t,h]
    v = jax.lax.dot_general(a, HsF, (((2,), (2,)), ((0, 1), (0, 1))))  # [F,b,H]
    return jnp.transpose(v, (1, 0, 2))     # [b,F,H]


def _head_local(v, demo, demoW, demob, mWq, mbq, mWk, mbk, mWv, mbv, mWo, mbo,
                ln1_a, ln1_b, ln2_a, ln2_b, ffn_W1, ffn_b1, ffn_W2, ffn_b2,
                fWq, fbq, fWk, fbk, fWv, fbv, outW, outb):
    """MHA + DeCov + FFN + final attention. v: [b,F,H] local shard.

    Cross-core terms (DeCov mean/cov) use jax.lax.psum over axis 'b'.
    """
    Bs = v.shape[0]

    demo_main = jnp.tanh(_linear(demo, demoW, demob))[:, None, :]
    posi = jnp.concatenate([v, demo_main], axis=1)  # [b,F+1,H]

    xn = _layernorm(posi, ln1_a, ln1_b)
    N = xn.shape[1]
    dk = DM // NH
    qh = _linear(xn, mWq, mbq).reshape(Bs, N, NH, dk).transpose(0, 2, 1, 3)
    kh = _linear(xn, mWk, mbk).reshape(Bs, N, NH, dk).transpose(0, 2, 1, 3)
    vh = _linear(xn, mWv, mbv).reshape(Bs, N, NH, dk).transpose(0, 2, 1, 3)
    scores = jnp.einsum('bhnd,bhmd->bhnm', qh, kh) / jnp.sqrt(jnp.asarray(dk, v.dtype))
    p = jax.nn.softmax(scores, axis=-1)
    ctx = jnp.einsum('bhnm,bhmd->bhnd', p, vh).transpose(0, 2, 1, 3).reshape(Bs, N, DM)
    mha_out = _linear(ctx, mWo, mbo)

    # DeCov across the FULL batch: all-reduce the mean and the covariance.
    s1 = jax.lax.psum(mha_out.sum(axis=0), 'b')          # [N,DM]
    mean = s1 / B
    xc = mha_out - mean[None]
    xcN = jnp.transpose(xc, (1, 0, 2))                   # [N, b, DM]
    cov_l = jax.lax.dot_general(xcN, xcN, (((1,), (1,)), ((0,), (0,))))
    cov = jax.lax.psum(cov_l, 'b') / (B - 1.0)           # [N, DM, DM]
    eye = jnp.eye(DM, dtype=cov.dtype)[None]
    decov = 0.5 * (jnp.sum(cov * cov) - jnp.sum((cov * eye) ** 2))

    h1 = posi + mha_out
    xn2 = _layernorm(h1, ln2_a, ln2_b)
    ffn = _linear(jax.nn.relu(_linear(xn2, ffn_W1, ffn_b1)), ffn_W2, ffn_b2)
    h2 = h1 + ffn

    fq = _linear(h2[:, -1, :], fWq, fbq)
    fk = _linear(h2, fWk, fbk)
    fv = _linear(h2, fWv, fbv)
    fe = jnp.einsum('bna,ba->bn', fk, fq)
    alpha = jax.nn.softmax(fe, axis=1)
    pooled = jnp.einsum('bn,bna->ba', alpha, fv)

    out = jax.nn.sigmoid(_linear(pooled, outW, outb))
    return out, decov


_COMPILED = None


def _get_compiled():
    global _COMPILED
    if _COMPILED is not None:
        return _COMPILED

    devs = jax.devices()[:NCORES]
    mesh = Mesh(np.asarray(devs), ('b',))

    shard_specs = tuple(P('b') if n in _SHARDED else P() for n in _ARGNAMES)

    def _smap(fn, in_specs, out_specs):
        try:
            return _shard_map_fn(fn, mesh=mesh, in_specs=in_specs,
                                 out_specs=out_specs, check_vma=False)
        except TypeError:
            return _shard_map_fn(fn, mesh=mesh, in_specs=in_specs,
                                 out_specs=out_specs, check_rep=False)

    gru_in = (P('b'), P(), P(), P(), P())
    jfn_gru = jax.jit(_smap(_gru_local, gru_in, P('b')))

    att_in = (P('b'), P(), P(), P())
    jfn_att = jax.jit(_smap(_att_local, att_in, P('b')))

    head_in = (P('b'),) + tuple(
        P('b') if n in _SHARDED else P() for n in _HEADNAMES
    )  # v sharded, demo sharded, weights replicated
    jfn_head = jax.jit(_smap(_head_local, head_in, (P('b'), P())))

    _COMPILED = (jfn_gru, jfn_att, jfn_head, mesh, shard_specs)
    return _COMPILED


def _run(jfn_gru, jfn_att, jfn_head, args):
    d = dict(zip(_ARGNAMES, args))
    Hs = jfn_gru(*[d[n] for n in _GRUNAMES])
    v = jfn_att(Hs, d['att_Wt'], d['att_Wx'], d['att_rate'])
    out, decov = jfn_head(v, *[d[n] for n in _HEADNAMES])
    return out, decov


def kernel(**inputs):
    jfn_gru, jfn_att, jfn_head, mesh, shard_specs = _get_compiled()
    args = []
    for name, spec in zip(_ARGNAMES, shard_specs):
        arr = jnp.asarray(inputs[name])
        args.append(jax.device_put(arr, NamedSharding(mesh, spec)))
    out, decov = _run(jfn_gru, jfn_att, jfn_head, args)
    out = np.asarray(jax.device_get(out)).astype(np.float32)
    decov = np.asarray(jax.device_get(decov)).astype(np.float32)
    return out, decov


if __name__ == '__main__':
    rng = np.random.default_rng(0)
    dummy = {
        'x': rng.standard_normal((B, T, F), dtype=np.float32),
        'demo': rng.standard_normal((B, DEMO), dtype=np.float32),
    }
    print('smoke test placeholder')


# revision 16
# speedup vs baseline: 3.5100x; 3.5100x over previous
"""ConCare forward pass distributed across 8 Trainium2 NeuronCores.

Sharding: data-parallel over batch B=64 (8 samples per core). The GRU scan,
per-feature attention, MHA and FFN are batch-local; DeCov's batch-mean and
covariance are the only cross-core terms (all-reduce via psum).
"""

import numpy as np
import jax
import jax.numpy as jnp
from jax.sharding import Mesh, PartitionSpec as P, NamedSharding

try:
    from jax import shard_map as _shard_map_fn  # jax >= 0.6
except ImportError:
    from jax.experimental.shard_map import shard_map as _shard_map_fn

B, T, F, H, A, DFF, DEMO, NH = 64, 96, 48, 128, 8, 512, 12, 4
DM = H
NCORES = 8

_ARGNAMES = [
    'x', 'demo', 'gru_Wih', 'gru_Whh', 'gru_bih', 'gru_bhh', 'att_Wt', 'att_Wx',
    'att_rate', 'demoW', 'demob', 'mWq', 'mbq', 'mWk', 'mbk', 'mWv', 'mbv',
    'mWo', 'mbo', 'ln1_a', 'ln1_b', 'ln2_a', 'ln2_b', 'ffn_W1', 'ffn_b1',
    'ffn_W2', 'ffn_b2', 'fWq', 'fbq', 'fWk', 'fbk', 'fWv', 'fbv', 'outW', 'outb',
]
_SHARDED = {'x', 'demo'}  # leading dim B; everything else replicated
_GRUNAMES = ['x', 'gru_Wih', 'gru_Whh', 'gru_bih', 'gru_bhh']
_ATTNAMES = ['att_Wt', 'att_Wx', 'att_rate']
_HEADNAMES = [n for n in _ARGNAMES
              if n not in _GRUNAMES and n not in _ATTNAMES]  # demo, mha, ffn, final


def _linear(x, W, b):
    return jnp.einsum('...i,oi->...o', x, W) + b


def _layernorm(x, a, b, eps=1e-7):
    mean = x.mean(-1, keepdims=True)
    std = jnp.std(x, axis=-1, keepdims=True, ddof=1)
    return a * (x - mean) / (std + eps) + b


def _gru_local(x, gru_Wih, gru_Whh, gru_bih, gru_bhh):
    """GRU scan on one core. x: [b,T,F] -> Hs [b,T,F,H]."""
    Bs = x.shape[0]
    Hd = gru_Whh.shape[-1]

    xT = jnp.transpose(x, (1, 2, 0))  # [T, F, b]

    def step(h, xt):  # h: [F,b,H], xt: [F,b]
        gi = xt[:, :, None] * gru_Wih[:, None, :] + gru_bih[:, None, :]  # [F,b,3H]
        gh = jax.lax.dot_general(h, gru_Whh, (((2,), (2,)), ((0,), (0,))))
        gh = gh + gru_bhh[:, None, :]  # [F,b,3H]
        ir, iz, inn = jnp.split(gi, 3, -1)
        hr, hz, hn = jnp.split(gh, 3, -1)
        r = jax.nn.sigmoid(ir + hr)
        z = jax.nn.sigmoid(iz + hz)
        n = jnp.tanh(inn + r * hn)
        hnew = (1.0 - z) * n + z * h
        return hnew, hnew

    h0 = jnp.zeros((F, Bs, Hd), x.dtype)
    _, Hs = jax.lax.scan(step, h0, xT, unroll=4)  # [T,F,b,H]
    return jnp.transpose(Hs, (2, 0, 1, 3))    # [b,T,F,H]


def _att_local(Hs, att_Wt, att_Wx, att_rate):
    """Per-feature time attention. Hs: [b,T,F,H] -> v [b,F,H]."""
    HsF = jnp.transpose(Hs, (2, 0, 1, 3))  # [F, b, T, H]
    hlast = HsF[:, :, -1, :]               # [F, b, H]

    # q[f,b,a] ; k[f,b,t,a]
    q = jax.lax.dot_general(hlast, att_Wt, (((2,), (1,)), ((0,), (0,))))
    HsFlat = HsF.reshape(F, -1, H)         # [F, b*T, H]
    k = jax.lax.dot_general(HsFlat, att_Wx, (((2,), (1,)), ((0,), (0,))))
    k = k.reshape(F, HsF.shape[1], T, A)   # [F, b, T, A]

    # dot[f,b,t] = sum_a q[f,b,a] * k[f,b,t,a]
    dot = jax.lax.dot_general(k, q, (((3,), (2,)), ((0, 1), (0, 1))))  # [F,b,T]
    b_time = jnp.arange(T, 0, -1, dtype=Hs.dtype)[None, None, :]
    sr = jax.nn.sigmoid(att_rate)[:, None, None]
    sd = jax.nn.sigmoid(dot)
    denom = sr * (jnp.log(2.72 + (1.0 - sd)) * b_time)
    e = jax.nn.relu(sd / denom)
    a = jax.nn.softmax(e, axis=2)          # softmax over t
    # v[f,b,h] = sum_t a[f,b,t] * Hs[f,b,t,h]
    v = jax.lax.dot_general(a, HsF, (((2,), (2,)), ((0, 1), (0, 1))))  # [F,b,H]
    return jnp.transpose(v, (1, 0, 2))     # [b,F,H]


def _head_local(v, demo, demoW, demob, mWq, mbq, mWk, mbk, mWv, mbv, mWo, mbo,
                ln1_a, ln1_b, ln2_a, ln2_b, ffn_W1, ffn_b1, ffn_W2, ffn_b2,
                fWq, fbq, fWk, fbk, fWv, fbv, outW, outb):
    """MHA + DeCov + FFN + final attention. v: [b,F,H] local shard.

    Cross-core terms (DeCov mean/cov) use jax.lax.psum over axis 'b'.
    """
    Bs = v.shape[0]

    demo_main = jnp.tanh(_linear(demo, demoW, demob))[:, None, :]
    posi = jnp.concatenate([v, demo_main], axis=1)  # [b,F+1,H]

    xn = _layernorm(posi, ln1_a, ln1_b)
    N = xn.shape[1]
    dk = DM // NH
    qh = _linear(xn, mWq, mbq).reshape(Bs, N, NH, dk).transpose(0, 2, 1, 3)
    kh = _linear(xn, mWk, mbk).reshape(Bs, N, NH, dk).transpose(0, 2, 1, 3)
    vh = _linear(xn, mWv, mbv).reshape(Bs, N, NH, dk).transpose(0, 2, 1, 3)
    scores = jnp.einsum('bhnd,bhmd->bhnm', qh, kh) / jnp.sqrt(jnp.asarray(dk, v.dtype))
    p = jax.nn.softmax(scores, axis=-1)
    ctx = jnp.einsum('bhnm,bhmd->bhnd', p, vh).transpose(0, 2, 1, 3).reshape(Bs, N, DM)
    mha_out = _linear(ctx, mWo, mbo)

    # DeCov across the FULL batch: all-reduce the mean and the covariance.
    s1 = jax.lax.psum(mha_out.sum(axis=0), 'b')          # [N,DM]
    mean = s1 / B
    xc = mha_out - mean[None]
    xcN = jnp.transpose(xc, (1, 0, 2))                   # [N, b, DM]
    cov_l = jax.lax.dot_general(xcN, xcN, (((1,), (1,)), ((0,), (0,))))
    cov = jax.lax.psum(cov_l, 'b') / (B - 1.0)           # [N, DM, DM]
    eye = jnp.eye(DM, dtype=cov.dtype)[None]
    decov = 0.5 * (jnp.sum(cov * cov) - jnp.sum((cov * eye) ** 2))

    h1 = posi + mha_out
    xn2 = _layernorm(h1, ln2_a, ln2_b)
    ffn = _linear(jax.nn.relu(_linear(xn2, ffn_W1, ffn_b1)), ffn_W2, ffn_b2)
    h2 = h1 + ffn

    fq = _linear(h2[:, -1, :], fWq, fbq)
    fk = _linear(h2, fWk, fbk)
    fv = _linear(h2, fWv, fbv)
    fe = jnp.einsum('bna,ba->bn', fk, fq)
    alpha = jax.nn.softmax(fe, axis=1)
    pooled = jnp.einsum('bn,bna->ba', alpha, fv)

    out = jax.nn.sigmoid(_linear(pooled, outW, outb))
    return out, decov


_COMPILED = None


def _get_compiled():
    global _COMPILED
    if _COMPILED is not None:
        return _COMPILED

    devs = jax.devices()[:NCORES]
    mesh = Mesh(np.asarray(devs), ('b',))

    shard_specs = tuple(P('b') if n in _SHARDED else P() for n in _ARGNAMES)

    def _smap(fn, in_specs, out_specs):
        try:
            return _shard_map_fn(fn, mesh=mesh, in_specs=in_specs,
                                 out_specs=out_specs, check_vma=False)
        except TypeError:
            return _shard_map_fn(fn, mesh=mesh, in_specs=in_specs,
                                 out_specs=out_specs, check_rep=False)

    gru_in = (P('b'), P(), P(), P(), P())
    jfn_gru = jax.jit(_smap(_gru_local, gru_in, P('b')))

    att_in = (P('b'), P(), P(), P())
    jfn_att = jax.jit(_smap(_att_local, att_in, P('b')))

    head_in = (P('b'),) + tuple(
        P('b') if n in _SHARDED else P() for n in _HEADNAMES
    )  # v sharded, demo sharded, weights replicated
    jfn_head = jax.jit(_smap(_head_local, head_in, (P('b'), P())))

    _COMPILED = (jfn_gru, jfn_att, jfn_head, mesh, shard_specs)
    return _COMPILED


def _run(jfn_gru, jfn_att, jfn_head, args):
    d = dict(zip(_ARGNAMES, args))
    Hs = jfn_gru(*[d[n] for n in _GRUNAMES])
    v = jfn_att(Hs, d['att_Wt'], d['att_Wx'], d['att_rate'])
    out, decov = jfn_head(v, *[d[n] for n in _HEADNAMES])
    return out, decov


def kernel(**inputs):
    jfn_gru, jfn_att, jfn_head, mesh, shard_specs = _get_compiled()
    args = []
    for name, spec in zip(_ARGNAMES, shard_specs):
        arr = jnp.asarray(inputs[name])
        args.append(jax.device_put(arr, NamedSharding(mesh, spec)))
    out, decov = _run(jfn_gru, jfn_att, jfn_head, args)
    out = np.asarray(jax.device_get(out)).astype(np.float32)
    decov = np.asarray(jax.device_get(decov)).astype(np.float32)
    return out, decov


if __name__ == '__main__':
    rng = np.random.default_rng(0)
    dummy = {
        'x': rng.standard_normal((B, T, F), dtype=np.float32),
        'demo': rng.standard_normal((B, DEMO), dtype=np.float32),
    }
    print('smoke test placeholder')
